# revision 3
# baseline (speedup 1.0000x reference)
"""Trainium2 Bass kernel for nn_GCNDecoder: GCN decoder over 128 independent
400-node graphs (64 avg degree), data-parallel over graphs across 8 cores.

Strategy (per core, 16 graphs):
  For each graph build the transposed weighted adjacency AT[src, dst] in PSUM
  by accumulating per-chunk (128-edge) one-hot outer products on the tensor
  engine; one-hots are built with fp16 4x-mode tensor_scalar compares against
  an iota row. A count histogram (dst>>7, dst&127) rides along. Both convs are
  then small dense matmuls against AT; the final 400x400 output is a 17-wide
  basis matmul, symmetrized in PSUM and pushed through the sigmoid LUT.

Self-contained: hardcodes shapes B=128, NN=400, FEAT=256, DEG=64, 8 cores.
"""
import numpy as np

import concourse.bass as bass
import concourse.mybir as mybir
import concourse.tile as tile
from concourse.bass_utils import run_bass_kernel_spmd

F32 = mybir.dt.float32
F16 = mybir.dt.float16
I32 = mybir.dt.int32
AL = mybir.AluOpType

B, NN, FEAT, DEG = 128, 400, 256, 64
NCORES = 8
NG = B // NCORES           # graphs per core
EPG = NN * DEG             # edges per graph
NCHUNK = EPG // 128        # 200 edge chunks per graph
NBLK = 4                   # 128-node blocks (512 padded)

_ES_CTR = [0]


def _split_excess_waits(nc, max_keep=1):
    """walrus in this environment rejects instructions with more than ~2 sync
    commands; keep at most one wait per instruction and push the rest onto
    preceding EventSemaphore instructions on the same engine."""
    for f in nc.m.functions:
        for bb in f.blocks:
            changed = False
            new = []
            for ins in bb.instructions:
                si = ins.sync_info
                if si is None or len(si.on_wait) <= max_keep:
                    new.append(ins)
                    continue
                waits = list(si.on_wait)
                for w in waits[:-max_keep]:
                    _ES_CTR[0] += 1
                    es = mybir.InstEventSemaphore(name=f"wsplit-{_ES_CTR[0]}")
                    es.engine = ins.engine
                    es.sync_info = mybir.SyncInfo(on_wait=[w], on_update=[])
                    new.append(es)
                si.on_wait = waits[-max_keep:]
                changed = True
                new.append(ins)
            if changed:
                bb.instructions = new


def _build(ng):
    nc = bass.Bass()
    xT = nc.declare_dram_parameter("xT", [256, ng], F32, isOutput=False)
    fc1_wT = nc.declare_dram_parameter("fc1_wT", [256, 512], F32, isOutput=False)
    fc1_b_rep = nc.declare_dram_parameter("fc1_b_rep", [ng, 512], F32, isOutput=False)
    src_t = nc.declare_dram_parameter("src_t", [ng, 128, NCHUNK], I32, isOutput=False)
    dst_t = nc.declare_dram_parameter("dst_t", [ng, 128, NCHUNK], I32, isOutput=False)
    ew_t = nc.declare_dram_parameter("ew_t", [ng, 128, NCHUNK], F32, isOutput=False)
    iota512 = nc.declare_dram_parameter("iota512", [128, 512], F16, isOutput=False)
    ident = nc.declare_dram_parameter("identity", [128, 128], F32, isOutput=False)
    wr1_rep = nc.declare_dram_parameter("wr1_rep", [128, 8], F32, isOutput=False)
    wo1_rep = nc.declare_dram_parameter("wo1_rep", [128, 8], F32, isOutput=False)
    br1_rep = nc.declare_dram_parameter("br1_rep", [128, 8], F32, isOutput=False)
    vext = nc.declare_dram_parameter("vext", [17, 400], F32, isOutput=False)
    y = nc.declare_dram_parameter("y", [ng, NN, NN], F32, isOutput=True)

    with tile.TileContext(nc) as tc:
        with (
            tc.tile_pool(name="const", bufs=1) as constp,
            tc.tile_pool(name="hpool", bufs=1) as hpool,
            tc.tile_pool(name="edges", bufs=2) as edgep,
            tc.tile_pool(name="oh", bufs=4) as ohp,
            tc.tile_pool(name="at", bufs=2) as atp,
            tc.tile_pool(name="small", bufs=2) as smallp,
            tc.tile_pool(name="gt", bufs=2) as gtp,
            tc.tile_pool(name="outp", bufs=3) as outp,
            tc.tile_pool(name="psA", bufs=1, space="PSUM") as psA,
            tc.tile_pool(name="psS", bufs=1, space="PSUM") as psS,
            tc.tile_pool(name="psO", bufs=1, space="PSUM") as psO,
        ):
            iota_sb = constp.tile([128, 512], F16)
            nc.sync.dma_start(out=iota_sb[:], in_=iota512[:])
            id_sb = constp.tile([128, 128], F32)
            nc.sync.dma_start(out=id_sb[:], in_=ident[:])
            wr1_sb = constp.tile([128, 8], F32)
            nc.sync.dma_start(out=wr1_sb[:], in_=wr1_rep[:])
            wo1_sb = constp.tile([128, 8], F32)
            nc.sync.dma_start(out=wo1_sb[:], in_=wo1_rep[:])
            br1_sb = constp.tile([128, 8], F32)
            nc.sync.dma_start(out=br1_sb[:], in_=br1_rep[:])
            vext_sb = constp.tile([17, 400], F32)
            nc.sync.dma_start(out=vext_sb[:], in_=vext[:])

            # H = lrelu(x @ fc1_w.T + b): [ng, 512]
            xT_sb = hpool.tile([128, 2, ng], F32)
            nc.sync.dma_start(out=xT_sb[:], in_=xT[:].rearrange("(c p) g -> p c g", p=128))
            w_sb = hpool.tile([128, 2, 512], F32)
            nc.sync.dma_start(out=w_sb[:], in_=fc1_wT[:].rearrange("(c p) n -> p c n", p=128))
            b_sb = hpool.tile([ng, 512], F32)
            nc.sync.dma_start(out=b_sb[:], in_=fc1_b_rep[:])
            h_ps = psS.tile([ng, 512], F32, space="PSUM", tag="pss")
            for c in range(2):
                nc.tensor.matmul(
                    out=h_ps[:], lhsT=xT_sb[:, c, :], rhs=w_sb[:, c, :],
                    start=(c == 0), stop=(c == 1),
                )
            h_sb = hpool.tile([ng, 512], F32)
            nc.vector.tensor_tensor(out=h_sb[:], in0=h_ps[:], in1=b_sb[:], op=AL.add)
            h_neg = hpool.tile([ng, 512], F32)
            nc.vector.tensor_scalar_mul(h_neg[:], h_sb[:], 0.01)
            nc.vector.tensor_tensor(out=h_sb[:], in0=h_sb[:], in1=h_neg[:], op=AL.max)
            ht_sb = hpool.tile([128, NBLK, ng], F32)
            for ib in range(NBLK):
                ht_ps = psS.tile([128, ng], F32, space="PSUM", tag="pss")
                nc.tensor.transpose(
                    out=ht_ps[:], in_=h_sb[:, ib * 128:(ib + 1) * 128],
                    identity=id_sb[:ng, :ng],
                )
                nc.vector.tensor_copy(ht_sb[:, ib, :], ht_ps[:])

            for g in range(ng):
                src_i = edgep.tile([128, NCHUNK], I32, tag="srci")
                dst_i = edgep.tile([128, NCHUNK], I32, tag="dsti")
                ew_f = edgep.tile([128, NCHUNK], F32, tag="ewf")
                nc.sync.dma_start(out=src_i[:], in_=src_t[g])
                nc.sync.dma_start(out=dst_i[:], in_=dst_t[g])
                nc.sync.dma_start(out=ew_f[:], in_=ew_t[g])
                srcf = edgep.tile([128, NCHUNK], F32, tag="srcf")
                nc.vector.tensor_copy(srcf[:], src_i[:])
                dstf = edgep.tile([128, NCHUNK], F32, tag="dstf")
                nc.vector.tensor_copy(dstf[:], dst_i[:])
                dhi_i = edgep.tile([128, NCHUNK], I32, tag="dhii")
                nc.vector.tensor_scalar(dhi_i[:], dst_i[:], 7, None, AL.logical_shift_right)
                dlo_i = edgep.tile([128, NCHUNK], I32, tag="dloi")
                nc.vector.tensor_scalar(dlo_i[:], dst_i[:], 127, None, AL.bitwise_and)
                dhif = edgep.tile([128, NCHUNK], F32, tag="dhif")
                nc.vector.tensor_copy(dhif[:], dhi_i[:])
                dlof = edgep.tile([128, NCHUNK], F32, tag="dlof")
                nc.vector.tensor_copy(dlof[:], dlo_i[:])

                at_ps = [
                    psA.tile([128, 400], F32, space="PSUM", tag=f"at{sb}", name=f"at{sb}")
                    for sb in range(NBLK)
                ]
                cnt_ps = psO.tile([4, 128], F32, space="PSUM", tag="cnt")
                for t in range(NCHUNK):
                    ohsrc = ohp.tile([128, 512], F16, tag="ohsrc")
                    nc.vector.tensor_scalar(
                        ohsrc[:], iota_sb[:], srcf[:, t:t + 1], None, AL.is_equal
                    )
                    ohdst = ohp.tile([128, 512], F16, tag="ohdst")
                    nc.vector.tensor_scalar(
                        ohdst[:], iota_sb[:], dstf[:, t:t + 1], ew_f[:, t:t + 1],
                        AL.is_equal, AL.mult,
                    )
                    ublk = ohp.tile([128, 4], F16, tag="ublk")
                    nc.vector.tensor_scalar(
                        ublk[:], iota_sb[:, :4], dhif[:, t:t + 1], None, AL.is_equal
                    )
                    vdll = ohp.tile([128, 128], F16, tag="vdll")
                    nc.vector.tensor_scalar(
                        vdll[:], iota_sb[:, :128], dlof[:, t:t + 1], None, AL.is_equal
                    )
                    first, last = (t == 0), (t == NCHUNK - 1)
                    for sb in range(NBLK):
                        nc.tensor.matmul(
                            out=at_ps[sb][:],
                            lhsT=ohsrc[:, sb * 128:(sb + 1) * 128],
                            rhs=ohdst[:, :400],
                            start=first, stop=last,
                        )
                    nc.tensor.matmul(
                        out=cnt_ps[:], lhsT=ublk[:], rhs=vdll[:],
                        start=first, stop=last,
                    )
                at_sb = [
                    atp.tile([128, 512], F32, tag=f"atsb{sb}", name=f"atsb{sb}")
                    for sb in range(NBLK)
                ]
                for sb in range(NBLK):
                    nc.vector.memset(at_sb[sb][:, 400:], 0.0)
                    nc.scalar.copy(out=at_sb[sb][:, :400], in_=at_ps[sb][:])
                cnt_sb = smallp.tile([4, 128], F32, tag="cntsb")
                nc.vector.tensor_copy(cnt_sb[:], cnt_ps[:])
                cntT_ps = psS.tile([128, 4], F32, space="PSUM", tag="pss")
                nc.tensor.transpose(out=cntT_ps[:], in_=cnt_sb[:], identity=id_sb[:4, :4])
                rcnt = smallp.tile([128, 4], F32, tag="rcnt")
                nc.vector.tensor_scalar(rcnt[:], cntT_ps[:], 1.0, None, AL.max)
                nc.vector.reciprocal(rcnt[:], rcnt[:])

                h2_sb = smallp.tile([128, NBLK, 8], F32, tag="h2")
                for ib in range(NBLK):
                    s1_ps = psS.tile([128, 1], F32, space="PSUM", tag="pss")
                    for sb in range(NBLK):
                        nc.tensor.matmul(
                            out=s1_ps[:],
                            lhsT=at_sb[sb][:, ib * 128:(ib + 1) * 128],
                            rhs=ht_sb[:, sb, g:g + 1],
                            start=(sb == 0), stop=(sb == NBLK - 1),
                        )
                    agg1 = smallp.tile([128, 1], F32, tag="agg1")
                    nc.vector.tensor_tensor(
                        out=agg1[:], in0=s1_ps[:], in1=rcnt[:, ib:ib + 1], op=AL.mult
                    )
                    z = smallp.tile([128, 8], F32, tag="z")
                    nc.vector.tensor_tensor(
                        out=z[:], in0=agg1[:].to_broadcast([128, 8]), in1=wr1_sb[:],
                        op=AL.mult,
                    )
                    z2 = smallp.tile([128, 8], F32, tag="z2")
                    nc.vector.tensor_tensor(
                        out=z2[:], in0=ht_sb[:, ib, g:g + 1].to_broadcast([128, 8]),
                        in1=wo1_sb[:], op=AL.mult,
                    )
                    nc.vector.tensor_tensor(out=z[:], in0=z[:], in1=z2[:], op=AL.add)
                    nc.vector.tensor_tensor(out=z[:], in0=z[:], in1=br1_sb[:], op=AL.add)
                    nc.vector.tensor_scalar_mul(z2[:], z[:], 0.01)
                    nc.vector.tensor_tensor(
                        out=h2_sb[:, ib, :], in0=z[:], in1=z2[:], op=AL.max
                    )

                gt_sb = gtp.tile([17, 400], F32, tag="gt")
                for ib in range(NBLK):
                    s2_ps = psS.tile([128, 8], F32, space="PSUM", tag="pss")
                    for sb in range(NBLK):
                        nc.tensor.matmul(
                            out=s2_ps[:],
                            lhsT=at_sb[sb][:, ib * 128:(ib + 1) * 128],
                            rhs=h2_sb[:, sb, :],
                            start=(sb == 0), stop=(sb == NBLK - 1),
                        )
                    gblk = smallp.tile([128, 17], F32, tag="gblk")
                    nc.vector.tensor_tensor(
                        out=gblk[:, 0:8], in0=s2_ps[:],
                        in1=rcnt[:, ib:ib + 1].to_broadcast([128, 8]), op=AL.mult,
                    )
                    nc.vector.tensor_copy(gblk[:, 8:16], h2_sb[:, ib, :])
                    nc.vector.memset(gblk[:, 16:17], 1.0)
                    ncols = 400 - ib * 128 if ib == NBLK - 1 else 128
                    t_ps = psS.tile([17, 128], F32, space="PSUM", tag="pss")
                    nc.tensor.transpose(out=t_ps[:], in_=gblk[:], identity=id_sb[:])
                    nc.vector.tensor_copy(
                        gt_sb[:, ib * 128:ib * 128 + ncols], t_ps[:, :ncols]
                    )

                for ib in range(NBLK):
                    nrows = 400 - ib * 128 if ib == NBLK - 1 else 128
                    p1 = psO.tile([128, 400], F32, space="PSUM", tag="p1")
                    nc.tensor.matmul(
                        out=p1[:nrows, :],
                        lhsT=gt_sb[:, ib * 128:ib * 128 + nrows],
                        rhs=vext_sb[:], start=True, stop=False,
                    )
                    nc.tensor.matmul(
                        out=p1[:nrows, :],
                        lhsT=vext_sb[:, ib * 128:ib * 128 + nrows],
                        rhs=gt_sb[:], start=False, stop=True,
                    )
                    o_sb = outp.tile([128, 400], F32, tag="osb")
                    nc.scalar.activation(
                        out=o_sb[:nrows, :], in_=p1[:nrows, :],
                        func=mybir.ActivationFunctionType.Sigmoid, scale=0.5,
                    )
                    nc.sync.dma_start(
                        out=y[g, ib * 128:ib * 128 + nrows, :], in_=o_sb[:nrows, :]
                    )
    _split_excess_waits(nc)
    return nc


TRACE = False
TRACE_TMPDIR = None

_NC_CACHE = {}


def _get_nc():
    if "nc" not in _NC_CACHE:
        _NC_CACHE["nc"] = _build(NG)
    return _NC_CACHE["nc"]


def kernel(x, edge_index, edge_attr, fc1_w, fc1_b, W_rel1, b_rel1, W_root1,
           W_rel2, b_rel2, W_root2):
    x = np.asarray(x, np.float32)
    edge_index = np.asarray(edge_index, np.int32)
    edge_attr = np.asarray(edge_attr, np.float32)
    fc1_w = np.asarray(fc1_w, np.float32)
    fc1_b = np.asarray(fc1_b, np.float32)
    W_rel1 = np.asarray(W_rel1, np.float32)
    b_rel1 = np.asarray(b_rel1, np.float32)
    W_root1 = np.asarray(W_root1, np.float32)
    W_rel2 = np.asarray(W_rel2, np.float32)
    b_rel2 = np.asarray(b_rel2, np.float32)
    W_root2 = np.asarray(W_root2, np.float32)

    # shard: graphs b -> core b // NG; edges of graph b are the contiguous
    # slice [b*EPG, (b+1)*EPG); indices re-based to graph-local.
    src = edge_index[0].reshape(B, EPG)
    dst = edge_index[1].reshape(B, EPG)
    base = (np.arange(B, dtype=np.int32) * NN)[:, None]
    src_loc = (src - base).astype(np.int32)
    dst_loc = (dst - base).astype(np.int32)
    ew = edge_attr.reshape(B, EPG)

    fc1_wT = np.zeros((256, 512), np.float32)
    fc1_wT[:, :NN] = fc1_w.T
    shared = {
        "fc1_wT": fc1_wT,
        "fc1_b_rep": np.tile(
            np.pad(fc1_b, (0, 512 - NN)).astype(np.float32)[None, :], (NG, 1)),
        "iota512": np.tile(np.arange(512, dtype=np.float16)[None, :], (128, 1)),
        "identity": np.eye(128, dtype=np.float32),
        "wr1_rep": np.tile(W_rel1[:, 0][None, :], (128, 1)).astype(np.float32),
        "wo1_rep": np.tile(W_root1[:, 0][None, :], (128, 1)).astype(np.float32),
        "br1_rep": np.tile(b_rel1[None, :], (128, 1)).astype(np.float32),
        "vext": np.concatenate(
            [W_rel2.T, W_root2.T, b_rel2[None, :]], axis=0).astype(np.float32),
    }
    in_maps = []
    for c in range(NCORES):
        gs = slice(c * NG, (c + 1) * NG)
        m = dict(shared)
        m["xT"] = np.ascontiguousarray(x[gs].T)
        m["src_t"] = np.ascontiguousarray(
            src_loc[gs].reshape(NG, NCHUNK, 128).transpose(0, 2, 1))
        m["dst_t"] = np.ascontiguousarray(
            dst_loc[gs].reshape(NG, NCHUNK, 128).transpose(0, 2, 1))
        m["ew_t"] = np.ascontiguousarray(
            ew[gs].reshape(NG, NCHUNK, 128).transpose(0, 2, 1))
        in_maps.append(m)

    nc = _get_nc()
    kw = {}
    if TRACE:
        kw = {"trace": True, "tmpdir": TRACE_TMPDIR}
    res = run_bass_kernel_spmd(nc, in_maps, list(range(NCORES)), **kw)
    kernel.last_exec_time_ns = res.exec_time_ns
    out = np.empty((B, NN, NN), np.float32)
    for c in range(NCORES):
        out[c * NG:(c + 1) * NG] = res.results[c]["y"]
    return out
